# revision 1
# baseline (speedup 1.0000x reference)
"""Trainium2 Bass kernel: batched complex-waveform similarity.

Math: reference computes
    bank = ifft_ortho(freq)                # [T, L] complex
    score = rx @ conj(bank).T              # [B, T] complex
    sim   = (score.re^2 + score.im^2) / temperature

Since the ortho DFT is unitary,  score = fft_ortho(rx) @ conj(freq).T.
So the kernel never builds the bank: it DFTs rx via a 128x128 matmul
(exact fp32), then runs one big complex GEMM [B,L]x[L,T] in bf16 with
fp32 PSUM accumulation, and a fused squared-magnitude epilogue.

Sharding: data-parallel over the rx batch dim across 8 NeuronCores;
freq (as a transposed bf16 [L, T] pair) is replicated on every core.

Per-core engine pipeline:
  PE   : DFT (fp32) + 512 bf16 matmuls [128,128]@[128,512] -> PSUM Sr/Si
  ACT  : t2 = Square(Si)                    (PSUM -> SBUF)
  DVE  : out = (Sr^2 + t2) * (1/temp)       (custom fused DVE op)
  SP   : HWDGE DMAs in/out
"""

import numpy as np
import ml_dtypes

B = 8192
T = 8192
L = 128
NCORES = 8
BPC = B // NCORES  # batch rows per core

_BF16 = ml_dtypes.bfloat16

_CACHE = {}


# --------------------------------------------------------------------------- #
# Custom DVE op: out = (Src0^2 + Src1) * C0
# (Src0 = Sr from PSUM, Src1 = Si^2 staged by ScalarE, C0 = 1/temperature)
# --------------------------------------------------------------------------- #
def _get_sqadd_op():
    import concourse.dve_ops as dve_ops
    from concourse.dve_spec import Spec, Src0, Src1, C0, sq, lower, _has_src1
    from concourse.dve_uop import DveOpSpec

    name = "SQ_ADD_SCALE_ANT"
    for op in dve_ops.OPS:
        if op.name == name:
            return op

    spec = Spec(
        body=(sq(Src0) + Src1) * C0,
        reference=lambda in0, in1, s0, s1, imm2: (
            (in0.astype(np.float32) ** 2 + in1.astype(np.float32)) * s0
        ).astype(np.float32),
    )
    opcode = dve_ops._CUSTOM_DVE_ROW_BASE + len(dve_ops.OPS)
    assert opcode < 0x20
    shas = {}
    for ver in ("v3", "v4"):
        compiled = DveOpSpec(
            name=name, opcode=opcode, uops=lower(spec, ver=ver), rd1_en=_has_src1(spec)
        )
        shas[ver] = compiled.sha(ver)
    op = dve_ops.DveOp(name, spec, subdim=False, uops_sha=shas)
    dve_ops.OPS.append(op)
    dve_ops.CUSTOM_DVE_SPECS[name] = spec
    dve_ops._SUB_OPCODE_FOR_NAME[name] = opcode
    return op


# --------------------------------------------------------------------------- #
# Bass program (one SPMD NeuronCore)
# --------------------------------------------------------------------------- #
def build_nc(bpc=BPC, t=T, debug=False):
    from contextlib import ExitStack

    import concourse.bacc as bacc
    import concourse.bass as bass
    import concourse.mybir as mybir
    import concourse.tile as tile

    f32 = mybir.dt.float32
    bf16 = mybir.dt.bfloat16
    sqadd = _get_sqadd_op()

    NG = 512  # output columns per PSUM group (1 bank)
    FG = 1024  # freq columns per SBUF tile / DMA
    assert bpc % 128 == 0 and t % FG == 0

    nc = bacc.Bacc("TRN2", target_bir_lowering=False, debug=debug, num_devices=NCORES)

    f32r = mybir.dt.float32r
    rxt_r = nc.dram_tensor("rxt_r", [L, bpc], f32r, kind="ExternalInput")
    rxt_i = nc.dram_tensor("rxt_i", [L, bpc], f32r, kind="ExternalInput")
    fqt_r = nc.dram_tensor("fqt_r", [L, t], bf16, kind="ExternalInput")
    fqt_i = nc.dram_tensor("fqt_i", [L, t], bf16, kind="ExternalInput")
    w_r = nc.dram_tensor("w_r", [L, L], f32r, kind="ExternalInput")
    w_i = nc.dram_tensor("w_i", [L, L], f32r, kind="ExternalInput")
    w_ni = nc.dram_tensor("w_ni", [L, L], f32r, kind="ExternalInput")
    temp = nc.dram_tensor("temp", [128, 1], f32, kind="ExternalInput")
    out = nc.dram_tensor("out", [bpc, t], f32, kind="ExternalOutput")

    with tile.TileContext(nc) as tc, ExitStack() as ctx:
        consts = ctx.enter_context(tc.tile_pool(name="consts", bufs=1))
        psum = ctx.enter_context(
            tc.tile_pool(name="psum", bufs=4, space=bass.MemorySpace.PSUM)
        )
        sq_pool = ctx.enter_context(tc.tile_pool(name="sq", bufs=6))
        out_pool = ctx.enter_context(tc.tile_pool(name="ob", bufs=6))

        # ---- PE warmup ------------------------------------------------ #
        # Dependency-free matmuls run during the input-DMA window so the
        # HAM clock gate is already at 8/8 when the real matmuls start.
        # Sized to finish just before rx lands (~16us): ~56 * ~110ns + ramp.
        warm_w = consts.tile([128, 128], bf16)
        nc.gpsimd.memset(warm_w[:], 0)
        warm_ps = psum.tile([128, NG], mybir.dt.float32, tag="si")
        for _ in range(48):
            nc.tensor.matmul(warm_ps[:, 0:128], warm_w[:], warm_w[:], start=True, stop=True)

        # ---- load constants / inputs ---------------------------------- #
        # rx + DFT consts on the SP HWDGE ring; freq on the ScalarE ring so
        # the two streams don't queue behind each other at startup.
        rxr_sb = consts.tile([L, bpc], f32r)
        nc.sync.dma_start(rxr_sb[:], rxt_r[:, :])
        rxi_sb = consts.tile([L, bpc], f32r)
        nc.sync.dma_start(rxi_sb[:], rxt_i[:, :])
        wr_sb = consts.tile([L, L], f32r)
        nc.sync.dma_start(wr_sb[:], w_r[:, :])
        wni_sb = consts.tile([L, L], f32r)
        nc.sync.dma_start(wni_sb[:], w_ni[:, :])
        wi_sb = consts.tile([L, L], f32r)
        nc.sync.dma_start(wi_sb[:], w_i[:, :])
        temp_sb = consts.tile([128, 1], f32)
        nc.sync.dma_start(temp_sb[:], temp[:, :])
        # Per-group freq tiles so the first matmuls only wait on 1 MiB.
        # Group 0 rides the ScalarE HWDGE ring (parallel with rx on the SP
        # ring); the bulk queues on the SP ring BEHIND rx so rx lands first.
        fr_sb = []
        fi_sb = []
        for g in range(t // FG):
            gs = slice(g * FG, (g + 1) * FG)
            eng = nc.scalar if g == 0 else nc.sync
            ftr = consts.tile([L, FG], bf16, tag=f"fr{g}")
            eng.dma_start(ftr[:], fqt_r[:, gs])
            fti = consts.tile([L, FG], bf16, tag=f"fi{g}")
            eng.dma_start(fti[:], fqt_i[:, gs])
            fr_sb.append(ftr)
            fi_sb.append(fti)

        invt_sb = consts.tile([128, 1], f32)
        nc.vector.reciprocal(invt_sb[:], temp_sb[:])

        # ---- DFT of rx (fp32, exact): rxfT = W @ rxT ------------------ #
        # W symmetric, so PE's lhsT is W itself.
        # rxfT_r = Wr@rxT_r - Wi@rxT_i ; rxfT_i = Wr@rxT_i + Wi@rxT_r
        rxf_r = consts.tile([L, bpc], bf16)
        rxf_i = consts.tile([L, bpc], bf16)
        rxf_nr = consts.tile([L, bpc], bf16)  # -rxfT_r
        for c0 in range(0, bpc, 512):
            cw = min(512, bpc - c0)
            cs = slice(c0, c0 + cw)
            pr = psum.tile([128, NG], mybir.dt.float32, tag="sr")
            nc.tensor.matmul(pr[:, 0:cw], wr_sb[:], rxr_sb[:, cs], start=True, stop=False)
            nc.tensor.matmul(pr[:, 0:cw], wni_sb[:], rxi_sb[:, cs], start=False, stop=True)
            pi = psum.tile([128, NG], mybir.dt.float32, tag="si")
            nc.tensor.matmul(pi[:, 0:cw], wr_sb[:], rxi_sb[:, cs], start=True, stop=False)
            nc.tensor.matmul(pi[:, 0:cw], wi_sb[:], rxr_sb[:, cs], start=False, stop=True)
            # chunked casts: the first main matmuls only need the first
            # 128-column slice of rxf, so don't gate them on the full cast
            for k0 in range(0, cw, 256):
                ks = slice(c0 + k0, c0 + k0 + 256)
                kp = slice(k0, k0 + 256)
                nc.vector.tensor_copy(rxf_r[:, ks], pr[:, kp])
                nc.vector.tensor_copy(rxf_i[:, ks], pi[:, kp])
                nc.vector.tensor_scalar_mul(rxf_nr[:, ks], pr[:, kp], -1.0)

        # ---- main complex GEMM + fused |.|^2 epilogue ----------------- #
        # Sr = rxf_r.T @ fr + rxf_i.T @ fi
        # Si = rxf_i.T @ fr - rxf_r.T @ fi
        OBW = 1024  # out staging tile width: 2 groups per 512 KiB DMA
        for m in range(bpc // 128):
            ms = slice(m * 128, (m + 1) * 128)
            ob = None
            for n in range(t // NG):
                g, j = divmod(n, FG // NG)
                js = slice(j * NG, (j + 1) * NG)
                sr = psum.tile([128, NG], mybir.dt.float32, tag="sr")
                si = psum.tile([128, NG], mybir.dt.float32, tag="si")
                nc.tensor.matmul(sr[:], rxf_r[:, ms], fr_sb[g][:, js], start=True, stop=False)
                nc.tensor.matmul(sr[:], rxf_i[:, ms], fi_sb[g][:, js], start=False, stop=True)
                nc.tensor.matmul(si[:], rxf_i[:, ms], fr_sb[g][:, js], start=True, stop=False)
                nc.tensor.matmul(si[:], rxf_nr[:, ms], fi_sb[g][:, js], start=False, stop=True)
                t2 = sq_pool.tile([128, NG], f32)
                nc.scalar.square(t2[:], si[:])
                o = n % (OBW // NG)
                if o == 0:
                    ob = out_pool.tile([128, OBW], f32)
                nc.vector._custom_dve(
                    sqadd,
                    out=ob[:, o * NG : (o + 1) * NG],
                    in0=sr[:],
                    in1=t2[:],
                    s0=invt_sb[:],
                )
                if m == bpc // 128 - 1 and n >= t // NG - OBW // NG:
                    # final pair: per-group DMAs so the last transfer is
                    # smaller and the kernel-exit barrier waits less
                    nc.sync.dma_start(
                        out[ms, n * NG : (n + 1) * NG], ob[:, o * NG : (o + 1) * NG]
                    )
                elif o == OBW // NG - 1:
                    n0 = n - o
                    # alternate output DMAs across both HWDGE rings — the
                    # ScalarE ring is idle once the freq loads finish
                    pair = (m * (t // NG) + n) // (OBW // NG)
                    oeng = nc.scalar if pair % 4 == 0 else nc.sync
                    oeng.dma_start(out[ms, n0 * NG : n0 * NG + OBW], ob[:])

    nc.compile()
    return nc


def _host_prep(rx_real, rx_imag, freq_real, freq_imag, temperature, bpc=BPC, t=T):
    """Layout marshaling only: shard/transpose/cast inputs for the cores."""
    lk = np.outer(np.arange(L), np.arange(L)).astype(np.float64)
    w = np.exp(-2j * np.pi * lk / L) / np.sqrt(L)  # ortho DFT matrix (symmetric)
    w_r = np.ascontiguousarray(w.real.astype(np.float32))
    w_i = np.ascontiguousarray(w.imag.astype(np.float32))
    w_ni = np.ascontiguousarray(-w_i)

    fqt_r = np.ascontiguousarray(freq_real[:t].T.astype(_BF16))  # [L, T]
    fqt_i = np.ascontiguousarray(freq_imag[:t].T.astype(_BF16))
    temp_col = np.full((128, 1), np.asarray(temperature), np.float32)

    rxt_r = np.asarray(rx_real, np.float32).T  # [L, B]
    rxt_i = np.asarray(rx_imag, np.float32).T

    in_maps = []
    for c in range(NCORES):
        cs = slice(c * bpc, (c + 1) * bpc)
        in_maps.append(
            {
                "rxt_r": np.ascontiguousarray(rxt_r[:, cs]),
                "rxt_i": np.ascontiguousarray(rxt_i[:, cs]),
                "fqt_r": fqt_r,
                "fqt_i": fqt_i,
                "w_r": w_r,
                "w_i": w_i,
                "w_ni": w_ni,
                "temp": temp_col,
            }
        )
    return in_maps


def kernel(rx_real, rx_imag, freq_real, freq_imag, temperature):
    from concourse.bass_utils import run_bass_kernel_spmd

    if "nc" not in _CACHE:
        _CACHE["nc"] = build_nc()
    nc = _CACHE["nc"]

    in_maps = _host_prep(rx_real, rx_imag, freq_real, freq_imag, temperature)
    res = run_bass_kernel_spmd(nc, in_maps, core_ids=list(range(NCORES)))
    _CACHE["last_result"] = res
    return np.concatenate([r["out"] for r in res.results], axis=0)

